# revision 13
# baseline (speedup 1.0000x reference)
"""Fused pairwise-MLP kernel for Trainium2 (8 NeuronCores, SPMD data-parallel).

Computes log_q[i, j] = W3 @ gelu(W2 @ gelu(a[j] + b[i] + b1) + b2) + b3
with a = z1 @ W1a.T, b = z2 @ W1b.T  (W1 = [W1a | W1b]), N=1024, H=EMB=128.

Sharding: rows of i (z2) split across 8 cores, z1 + weights replicated
(host-side sharding; no collectives).

The two gelu passes (2 x 131072 128-partition columns per core) are the
arithmetic bottleneck: the ACT engine runs them at 1 elem/lane/cycle and
nothing else on the chip has a gelu table.  gelu1 therefore runs almost
entirely on the Vector engine as a runtime-registered custom DVE op that
evaluates, in one 8-ALU-stage pass over two slots (2048 cols, subdim
form, per-slot bias via PageIdx),
    y = x' + ((x'^2 + ct2)*x'^2 + ct3)*x'^2,   x' = sqrt(beta)*(a + bias)
which equals sqrt(beta)*2*gelu(x) for a beta-normalized deg-6 even fit
(the leading Horner coefficient is normalized to 1 so the three scalar
ports cover bias-step + two coefficients).  The W2 stationary for these
slots is pre-scaled by 0.5/sqrt(beta).  The NA widest-range slots per
core run on ACT's exact gelu (host permutes i-rows so they land on the
fixed ACT slot positions; permutation undone on output gather).

gelu2 runs entirely on ACT (PSUM input, b2 via the bias port) as
2048-wide pair instructions + a 1024 single per 3-slot PSUM v-ring
period.  W3 matmuls are emitted 3+ slots late and batched after the
next period's W2 legs so the in-order PE stream never head-of-line
blocks the v-ring round-trip (pair-gelu2 -> 2x W2 -> next pair).

The W3 dot uses 32 zero-padded stationary variants (w3 at column k) so
slot s = 32g+k lands at PSUM partition s of a single [128,1024]
accumulation block (tile_position group g, accumulating matmuls).  All
128 output rows are evacuated with 4 instructions (+b3) and 5 DMAs.
"""

import numpy as np

import concourse.bacc as bacc
import concourse.bass as bass
import concourse.tile as tile
import concourse.mybir as mybir
from concourse import bass_utils

import concourse.dve_ops as dve_ops
from concourse.dve_ops import DveOp, OPS
from concourse.dve_spec import (
    Spec, Src0, C0, C1, C2, C3, PageIdx, lower, _spill_c3_to_src1,
)
from concourse.dve_uop import DveOpSpec


def _register(name, spec, subdim):
    if name in dve_ops._SUB_OPCODE_FOR_NAME:
        return next(o for o in OPS if o.name == name)
    row = dve_ops._CUSTOM_DVE_ROW_BASE + len(OPS)
    dve_ops._SUB_OPCODE_FOR_NAME[name] = row
    shas = {}
    for ver in ("v3", "v4"):
        try:
            s = DveOpSpec(name=name, opcode=row, uops=lower(spec, ver=ver),
                          rd1_en=True)
            shas[ver] = s.sha(ver)
        except Exception:
            pass
    op = DveOp(name, spec, subdim=subdim, uops_sha=shas)
    OPS.append(op)
    dve_ops.CUSTOM_DVE_SPECS[name] = spec
    return op


def _gelu1_single_spec():
    # x = in0 + s0; u = x*x; out = ((s1*u + imm2)*u + c3)*u + x
    x = Src0 + C0
    u = x * x
    t = ((C1 * u + C2) * u + C3) * u
    body = _spill_c3_to_src1(t + x)

    def ref(in0, in1, s0, s1, imm2):
        xx = in0.astype(np.float32) + s0
        uu = xx * xx
        return ((s1 * uu + imm2) * uu + in1) * uu + xx

    return Spec(body=body, reference=ref)


def _gelu1_pair_spec():
    # in0 = [P, 2, N]; bias steps via PageIdx(C0, C1); leading coef = 1
    pg = PageIdx(C0, C1)
    x = Src0 + pg
    u = x * x
    t = ((u + C2) * u + C3) * u
    body = _spill_c3_to_src1(t + x)

    def ref(in0, in1, s0, s1, imm2):
        x = in0.astype(np.float32)
        S = int(np.prod(x.shape[1:-1]))
        x3 = x.reshape((x.shape[0], S, x.shape[-1]))
        idx = s0[:, None] if isinstance(s0, np.ndarray) else s0
        s1v = s1[:, None] if isinstance(s1, np.ndarray) else s1
        bias = idx + np.arange(S, dtype=np.float32)[None, :, None] * s1v
        xx = x3 + bias
        uu = xx * xx
        c3v = in1 if not isinstance(in1, np.ndarray) else in1.reshape(-1, 1, 1)
        y = ((uu + imm2) * uu + c3v) * uu + xx
        return y.reshape(in0.shape)

    return Spec(body=body, reference=ref)


GELU1_OP = _register("GELU1_EVEN6_ANT", _gelu1_single_spec(), subdim=False)
GELU1P_OP = _register("GELU1_PAIR_ANT", _gelu1_pair_spec(), subdim=True)

# deg-6 even fit of x*erf(x/sqrt(2)) on |x| <= 3.67 (density-weighted,
# x ~ N(0, 0.586)): coefs of u, u^2, u^3
CC = (0.7720335236204651, -0.09365603610221726, 0.00457457167839083)
BETA = CC[2] ** 0.4               # leading-coef normalization
SQB = float(np.sqrt(BETA))
CT2 = float(CC[1] / BETA ** 1.5)  # u'^2 coef after normalization
CT3 = float(CC[0] / SQB)          # u'^1 coef

N = 1024
EMB = 128
HID = 128
NCORES = 8
SH = N // NCORES  # i-slots per core
F32 = mybir.dt.float32
FP16 = mybir.dt.float16
GELU = mybir.ActivationFunctionType.Gelu
COPY = mybir.ActivationFunctionType.Copy

# slots whose gelu1 runs on ACT (exact); host routes widest-range i's
# here.  Gaps of 15 keep every DVE run even-length (pairable).
ACT_SLOTS = tuple(range(6, 112, 15))  # 8 slots: 6,21,...,111
DVE_SLOTS = tuple(s for s in range(SH) if s not in ACT_SLOTS)


def _build(b3val):
    nc = bacc.Bacc("TRN2", target_bir_lowering=False, debug=False)

    z1Tq_d = [
        nc.dram_tensor(f"z1Tq{q}", (EMB, 256), F32, kind="ExternalInput")
        for q in range(4)
    ]
    z2T_d = nc.dram_tensor("z2T", (EMB, SH), F32, kind="ExternalInput")
    w1aT_d = nc.dram_tensor("w1aT", (EMB, HID), F32, kind="ExternalInput")
    w1bT_d = nc.dram_tensor("w1bT", (EMB, HID), F32, kind="ExternalInput")
    w2T_d = nc.dram_tensor("w2T", (HID, HID), F32, kind="ExternalInput")
    w3v_d = nc.dram_tensor("w3v", (HID, 1024), F32, kind="ExternalInput")
    b1_d = nc.dram_tensor("b1", (HID,), F32, kind="ExternalInput")
    b2_d = nc.dram_tensor("b2", (HID,), F32, kind="ExternalInput")
    out_d = nc.dram_tensor("out", (SH, N), F32, kind="ExternalOutput")

    with tile.TileContext(nc) as tc:
        _body(tc, out_d, z1Tq_d, z2T_d, w1aT_d, w1bT_d, w2T_d, w3v_d,
              b1_d, b2_d, b3val)

    nc.compile()
    return nc


def _body(tc, out_d, z1Tq_d, z2T_d, w1aT_d, w1bT_d, w2T_d, w3v_d,
          b1_d, b2_d, b3val):
    nc = tc.nc
    with (
        tc.tile_pool(name="const", bufs=1) as const,
        tc.tile_pool(name="h1p", bufs=4) as h1p,
        tc.tile_pool(name="h2p", bufs=3) as h2p,
        tc.tile_pool(name="srows", bufs=1) as srows,
        tc.tile_pool(name="ringp", bufs=1, space="PSUM") as ringp,
    ):
        # ACT warms the gelu table as its very first instruction (no DMAs
        # ride the scalar queue at startup).
        tiny = const.tile([1, 1], F32)
        nc.vector.memset(tiny, 0.0)
        warm = const.tile([1, 1], F32)
        nc.scalar.activation(warm, tiny, GELU)

        # ---- input DMAs spread over sync / scalar / gpsimd queues ----
        w1aT_sb = const.tile([128, HID], F32)
        nc.sync.dma_start(out=w1aT_sb, in_=w1aT_d.ap())
        w1bT_sb = const.tile([128, HID], F32)
        nc.scalar.dma_start(out=w1bT_sb, in_=w1bT_d.ap())
        z2T_sb = const.tile([128, SH], F32)
        nc.sync.dma_start(out=z2T_sb, in_=z2T_d.ap())
        z1T_sb = const.tile([128, N], F32)
        for q, eng in enumerate((nc.sync, nc.scalar, nc.gpsimd, nc.scalar)):
            eng.dma_start(out=z1T_sb[:, q * 256:(q + 1) * 256], in_=z1Tq_d[q].ap())
        w2T_f = const.tile([128, HID], F32)
        nc.sync.dma_start(out=w2T_f, in_=w2T_d.ap())
        w3v_f = const.tile([128, 1024], F32)
        nc.gpsimd.dma_start(out=w3v_f, in_=w3v_d.ap())
        b1_sb = const.tile([128, 1], F32)
        nc.gpsimd.dma_start(out=b1_sb, in_=b1_d.ap().rearrange("(p o) -> p o", o=1))
        b2_sb = const.tile([128, 1], F32)
        nc.gpsimd.dma_start(out=b2_sb, in_=b2_d.ap().rearrange("(p o) -> p o", o=1))

        c3p_sb = const.tile([128, 1], F32)
        nc.gpsimd.memset(c3p_sb, CT3)

        # fp16 stationaries (w2T on the startup-idle ACT, w3v on Pool)
        w2T_full = const.tile([128, HID], FP16)
        nc.scalar.activation(w2T_full, w2T_f, COPY, bias=0.0)
        w2T_half = const.tile([128, HID], FP16)  # x(0.5/SQB): h1' = SQB*2*gelu
        nc.scalar.activation(w2T_half, w2T_f, COPY, bias=0.0, scale=0.5 / SQB)
        w3v_h = const.tile([128, 1024], FP16)
        nc.gpsimd.tensor_copy(w3v_h, w3v_f)

        # ---- PSUM: 3 v-slots + [128,1024] W3 accumulation block ----
        ring = ringp.tile([128, 4096], F32)
        VS = [ring[:, 0:1024], ring[:, 1024:2048], ring[:, 2048:3072]]
        w3blk = ring[:, 3072:4096]

        # ---- prologue: b_pp tiles, scaled duplicated a ----
        tpb = ring[:, 2048:2048 + SH]   # v-slot 2 region, freed before use
        nc.tensor.matmul(tpb, w1bT_sb, z2T_sb)
        b_pp_sc = const.tile([128, SH], F32)       # SQB*(b + b1)
        nc.vector.tensor_scalar(out=b_pp_sc, in0=tpb, scalar1=b1_sb[:, 0:1],
                                scalar2=SQB, op0=mybir.AluOpType.add,
                                op1=mybir.AluOpType.mult)
        b_pp = const.tile([128, SH], F32)          # b + b1 (ACT slots)
        nc.vector.tensor_scalar(out=b_pp, in0=tpb, scalar1=b1_sb[:, 0:1],
                                scalar2=None, op0=mybir.AluOpType.add)
        d_sc = const.tile([128, SH], F32)          # pair bias deltas
        nc.vector.tensor_tensor(out=d_sc[:, 0:SH - 1], in0=b_pp_sc[:, 1:SH],
                                in1=b_pp_sc[:, 0:SH - 1],
                                op=mybir.AluOpType.subtract)

        tpa = ring[:, 0:1024]
        for q in range(4):
            nc.tensor.matmul(tpa[:, q * 256:(q + 1) * 256], w1aT_sb,
                             z1T_sb[:, q * 256:(q + 1) * 256])
        a_dbl = const.tile([128, 2048], F32)       # SQB*a, twice
        nc.vector.tensor_scalar(out=a_dbl[:, 0:1024], in0=tpa, scalar1=SQB,
                                scalar2=None, op0=mybir.AluOpType.mult)
        nc.scalar.activation(a_dbl[:, 1024:2048], tpa, COPY, bias=0.0,
                             scale=SQB)

        # ---- steady state ----
        srow = srows.tile([128, N], F32)
        h1map = {}

        def pump_g1(upto):
            s = pump_g1.next
            while s < min(upto, SH):
                if s in ACT_SLOTS:
                    h1 = h1p.tile([128, N], FP16, tag="h1s", name="h1s", bufs=3)
                    nc.scalar.activation(h1, a_dbl[:, 0:1024], GELU,
                                         bias=b_pp[:, s:s + 1], scale=1.0 / SQB)
                    h1map[s] = (h1, 0)
                    s += 1
                elif s + 1 < SH and (s + 1) not in ACT_SLOTS:
                    h1 = h1p.tile([128, 2048], FP16, tag="h1d", name="h1d",
                                  bufs=6)
                    nc.vector._custom_dve(
                        GELU1P_OP,
                        out=h1[:, :].rearrange("p (s n) -> p s n", n=N),
                        in0=a_dbl[:, :].rearrange("p (s n) -> p s n", n=N),
                        in1=c3p_sb[:, 0:1],
                        s0=b_pp_sc[:, s:s + 1], s1=d_sc[:, s:s + 1], imm2=CT2)
                    h1map[s] = (h1, 0)
                    h1map[s + 1] = (h1, 1024)
                    s += 2
                else:
                    h1 = h1p.tile([128, N], FP16, tag="h1s", name="h1s", bufs=3)
                    nc.vector._custom_dve(
                        GELU1_OP, out=h1, in0=a_dbl[:, 0:1024],
                        in1=c3p_sb[:, 0:1],
                        s0=b_pp_sc[:, s:s + 1], s1=1.0, imm2=CT2)
                    h1map[s] = (h1, 0)
                    s += 1
            pump_g1.next = s

        pump_g1.next = 0

        def emit_w2(s):
            h1, off = h1map.pop(s)
            w2 = w2T_full if s in ACT_SLOTS else w2T_half
            vs = VS[s % 3]
            for h in range(2):
                nc.tensor.matmul(vs[:, h * 512:(h + 1) * 512], w2,
                                 h1[:, off + h * 512:off + (h + 1) * 512])

        def emit_w3(s, h2, off):
            g, k = divmod(s, 32)
            w3k = w3v_h[:, 32 * k:32 * k + 32]
            for h in range(2):
                nc.tensor.matmul(
                    w3blk[32 * g:32 * g + 32, h * 512:(h + 1) * 512],
                    w3k, h2[:, off + h * 512:off + (h + 1) * 512],
                    tile_position=(0, 32 * g),
                    start=(k == 0), stop=(k == 31), skip_group_check=True)

        h2q = []

        def emit_g2_pair(s0, s1):
            h2 = h2p.tile([128, 2048], FP16, tag="h2", name="h2", bufs=5)
            nc.scalar.activation(h2, ring[:, (s0 % 3) * 1024:(s0 % 3) * 1024 + 2048],
                                 GELU, bias=b2_sb[:, 0:1])
            h2q.append((s0, h2, 0))
            h2q.append((s1, h2, 1024))

        def emit_g2_single(s):
            h2 = h2p.tile([128, 1024], FP16, tag="h2s", name="h2s", bufs=4)
            nc.scalar.activation(h2, VS[s % 3], GELU, bias=b2_sb[:, 0:1])
            h2q.append((s, h2, 0))

        def evac(g):
            if g < 3:
                nc.vector.tensor_scalar(
                    out=srow[32 * g:32 * g + 32, :],
                    in0=w3blk[32 * g:32 * g + 32, :],
                    scalar1=b3val, scalar2=None, op0=mybir.AluOpType.add)
                nc.sync.dma_start(out=out_d.ap()[32 * g:32 * g + 32, :],
                                  in_=srow[32 * g:32 * g + 32, :])
            else:
                # final group on the (now idle) ACT engine; split the DMA
                nc.scalar.activation(srow[96:128, :], w3blk[96:128, :], COPY,
                                     bias=float(b3val))
                nc.sync.dma_start(out=out_d.ap()[96:112, :],
                                  in_=srow[96:112, :])
                nc.scalar.dma_start(out=out_d.ap()[112:128, :],
                                    in_=srow[112:128, :])

        next_evac = 0
        w3_done = -1
        for s in range(SH):
            pump_g1(s + 6)
            emit_w2(s)
            r = s % 3
            if r == 1:
                emit_g2_pair(s - 1, s)
            elif r == 2:
                emit_g2_single(s)
            # W3 batched after the W2 legs of the next pair (see docstring)
            if r == 1:
                while h2q and h2q[0][0] <= s - 3:
                    sl, h2, off = h2q.pop(0)
                    emit_w3(sl, h2, off)
                    w3_done = sl
            while w3_done >= 32 * next_evac + 31:
                evac(next_evac)
                next_evac += 1
        while h2q:
            sl, h2, off = h2q.pop(0)
            emit_w3(sl, h2, off)
            w3_done = sl
            while w3_done >= 32 * next_evac + 31:
                evac(next_evac)
                next_evac += 1


_NC_CACHE = {}


def make_in_maps(z1, z2, W1, b1, W2, b2, W3, b3):
    f = np.float32
    z1 = np.asarray(z1, dtype=f)
    z2 = np.asarray(z2, dtype=f)
    W1 = np.asarray(W1, dtype=f)
    b1 = np.ascontiguousarray(np.asarray(b1, dtype=f))
    W2 = np.asarray(W2, dtype=f)
    b2 = np.ascontiguousarray(np.asarray(b2, dtype=f))
    W3 = np.asarray(W3, dtype=f)
    b3 = np.ascontiguousarray(np.asarray(b3, dtype=f))

    z1T = np.ascontiguousarray(z1.T)
    z1Tq = {
        f"z1Tq{q}": np.ascontiguousarray(z1T[:, q * 256:(q + 1) * 256])
        for q in range(4)
    }
    w1aT = np.ascontiguousarray(W1[:, :EMB].T)
    w1bT = np.ascontiguousarray(W1[:, EMB:].T)
    w2T = np.ascontiguousarray(W2.T)
    w3v = np.zeros((HID, 1024), dtype=f)
    for k in range(32):
        w3v[:, 32 * k + k] = W3[0]

    # slot permutation: widest-range i's (per core) -> ACT_SLOTS
    a = z1 @ W1[:, :EMB].T            # host copy, scheduling only
    bb = z2 @ W1[:, EMB:].T + b1
    amin, amax = a.min(0), a.max(0)
    ximax = np.maximum(np.abs(amin[None, :] + bb), np.abs(amax[None, :] + bb)).max(1)

    maps, perms = [], []
    act_slots = list(ACT_SLOTS)
    dve_slots = list(DVE_SLOTS)
    for c in range(NCORES):
        loc = ximax[c * SH:(c + 1) * SH]
        order = np.argsort(-loc)
        perm = np.empty(SH, dtype=np.int64)
        perm[act_slots] = order[:len(act_slots)]
        perm[dve_slots] = order[len(act_slots):]
        perms.append(perm)
        z2c = z2[c * SH:(c + 1) * SH][perm]
        maps.append({
            **z1Tq,
            "z2T": np.ascontiguousarray(z2c.T),
            "w1aT": w1aT, "w1bT": w1bT, "w2T": w2T, "w3v": w3v,
            "b1": b1, "b2": b2,
        })
    return maps, perms


def gather_out(results, perms):
    blocks = []
    for c in range(NCORES):
        o = results[c]["out"]
        inv = np.empty(SH, dtype=np.int64)
        inv[perms[c]] = np.arange(SH)
        blocks.append(o[inv])
    return np.concatenate(blocks, axis=0)


def kernel(z1, z2, W1, b1, W2, b2, W3, b3):
    b3v = float(np.asarray(b3).reshape(-1)[0])
    key = round(b3v, 9)
    if key not in _NC_CACHE:
        _NC_CACHE[key] = _build(b3v)
    nc = _NC_CACHE[key]

    in_maps, perms = make_in_maps(z1, z2, W1, b1, W2, b2, W3, b3)
    res = bass_utils.run_bass_kernel_spmd(nc, in_maps, core_ids=list(range(NCORES)))
    return gather_out(res.results, perms)


if __name__ == "__main__":
    rng = np.random.default_rng(0)
    s1 = 1.0 / np.sqrt(2 * EMB)
    s2 = 1.0 / np.sqrt(HID)
    ins = dict(
        z1=rng.standard_normal((N, EMB), dtype=np.float32),
        z2=rng.standard_normal((N, EMB), dtype=np.float32),
        W1=rng.uniform(-s1, s1, (HID, 2 * EMB)).astype(np.float32),
        b1=rng.uniform(-s1, s1, (HID,)).astype(np.float32),
        W2=rng.uniform(-s2, s2, (HID, HID)).astype(np.float32),
        b2=rng.uniform(-s2, s2, (HID,)).astype(np.float32),
        W3=rng.uniform(-s2, s2, (1, HID)).astype(np.float32),
        b3=rng.uniform(-s2, s2, (1,)).astype(np.float32),
    )
    out = kernel(**ins)
    print("out", out.shape, out.dtype, out[:2, :4])


# revision 14
# speedup vs baseline: 1.0384x; 1.0384x over previous
"""Fused pairwise-MLP kernel for Trainium2 (8 NeuronCores, SPMD data-parallel).

Computes log_q[i, j] = W3 @ gelu(W2 @ gelu(a[j] + b[i] + b1) + b2) + b3
with a = z1 @ W1a.T, b = z2 @ W1b.T  (W1 = [W1a | W1b]), N=1024, H=EMB=128.

Sharding: rows of i (z2) split across 8 cores, z1 + weights replicated
(host-side sharding; no collectives).

The two gelu passes (2 x 131072 128-partition columns per core) are the
arithmetic bottleneck: the ACT engine runs them at 1 elem/lane/cycle and
nothing else on the chip has a gelu table.  gelu1 therefore runs almost
entirely on the Vector engine as a runtime-registered custom DVE op that
evaluates, in one 8-ALU-stage pass over two slots (2048 cols, subdim
form, per-slot bias via PageIdx),
    y = x' + ((x'^2 + ct2)*x'^2 + ct3)*x'^2,   x' = sqrt(beta)*(a + bias)
which equals sqrt(beta)*2*gelu(x) for a beta-normalized deg-6 even fit
(the leading Horner coefficient is normalized to 1 so the three scalar
ports cover bias-step + two coefficients).  The W2 stationary for these
slots is pre-scaled by 0.5/sqrt(beta).  The NA widest-range slots per
core run on ACT's exact gelu (host permutes i-rows so they land on the
fixed ACT slot positions; permutation undone on output gather).

gelu2 runs entirely on ACT (PSUM input, b2 via the bias port) as
2048-wide pair instructions + a 1024 single per 3-slot PSUM v-ring
period.  W3 matmuls are emitted 3+ slots late and batched after the
next period's W2 legs so the in-order PE stream never head-of-line
blocks the v-ring round-trip (pair-gelu2 -> 2x W2 -> next pair).

The W3 dot uses 32 zero-padded stationary variants (w3 at column k) so
slot s = 32g+k lands at PSUM partition s of a single [128,1024]
accumulation block (tile_position group g, accumulating matmuls).  All
128 output rows are evacuated with 4 instructions (+b3) and 5 DMAs.
"""

import numpy as np

import concourse.bacc as bacc
import concourse.bass as bass
import concourse.tile as tile
import concourse.mybir as mybir
from concourse import bass_utils

import concourse.dve_ops as dve_ops
from concourse.dve_ops import DveOp, OPS
from concourse.dve_spec import (
    Spec, Src0, C0, C1, C2, C3, PageIdx, lower, _spill_c3_to_src1,
)
from concourse.dve_uop import DveOpSpec


def _register(name, spec, subdim):
    if name in dve_ops._SUB_OPCODE_FOR_NAME:
        return next(o for o in OPS if o.name == name)
    row = dve_ops._CUSTOM_DVE_ROW_BASE + len(OPS)
    dve_ops._SUB_OPCODE_FOR_NAME[name] = row
    shas = {}
    for ver in ("v3", "v4"):
        try:
            s = DveOpSpec(name=name, opcode=row, uops=lower(spec, ver=ver),
                          rd1_en=True)
            shas[ver] = s.sha(ver)
        except Exception:
            pass
    op = DveOp(name, spec, subdim=subdim, uops_sha=shas)
    OPS.append(op)
    dve_ops.CUSTOM_DVE_SPECS[name] = spec
    return op


def _gelu1_single_spec():
    # x = in0 + s0; u = x*x; out = ((s1*u + imm2)*u + c3)*u + x
    x = Src0 + C0
    u = x * x
    t = ((C1 * u + C2) * u + C3) * u
    body = _spill_c3_to_src1(t + x)

    def ref(in0, in1, s0, s1, imm2):
        xx = in0.astype(np.float32) + s0
        uu = xx * xx
        return ((s1 * uu + imm2) * uu + in1) * uu + xx

    return Spec(body=body, reference=ref)


def _gelu1_pair_spec():
    # in0 = [P, 2, N]; bias steps via PageIdx(C0, C1); leading coef = 1
    pg = PageIdx(C0, C1)
    x = Src0 + pg
    u = x * x
    t = ((u + C2) * u + C3) * u
    body = _spill_c3_to_src1(t + x)

    def ref(in0, in1, s0, s1, imm2):
        x = in0.astype(np.float32)
        S = int(np.prod(x.shape[1:-1]))
        x3 = x.reshape((x.shape[0], S, x.shape[-1]))
        idx = s0[:, None] if isinstance(s0, np.ndarray) else s0
        s1v = s1[:, None] if isinstance(s1, np.ndarray) else s1
        bias = idx + np.arange(S, dtype=np.float32)[None, :, None] * s1v
        xx = x3 + bias
        uu = xx * xx
        c3v = in1 if not isinstance(in1, np.ndarray) else in1.reshape(-1, 1, 1)
        y = ((uu + imm2) * uu + c3v) * uu + xx
        return y.reshape(in0.shape)

    return Spec(body=body, reference=ref)


GELU1_OP = _register("GELU1_EVEN6_ANT", _gelu1_single_spec(), subdim=False)
GELU1P_OP = _register("GELU1_PAIR_ANT", _gelu1_pair_spec(), subdim=True)

# deg-6 even fit of x*erf(x/sqrt(2)) on |x| <= 3.67 (density-weighted,
# x ~ N(0, 0.586)): coefs of u, u^2, u^3
CC = (0.7720335236204651, -0.09365603610221726, 0.00457457167839083)
BETA = CC[2] ** 0.4               # leading-coef normalization
SQB = float(np.sqrt(BETA))
CT2 = float(CC[1] / BETA ** 1.5)  # u'^2 coef after normalization
CT3 = float(CC[0] / SQB)          # u'^1 coef

N = 1024
EMB = 128
HID = 128
NCORES = 8
SH = N // NCORES  # i-slots per core
F32 = mybir.dt.float32
FP16 = mybir.dt.float16
GELU = mybir.ActivationFunctionType.Gelu
COPY = mybir.ActivationFunctionType.Copy

# slots whose gelu1 runs on ACT (exact); host routes widest-range i's
# here.  Gaps of 15 keep every DVE run even-length (pairable).
ACT_SLOTS = tuple(range(6, 112, 15))  # 8 slots: 6,21,...,111
DVE_SLOTS = tuple(s for s in range(SH) if s not in ACT_SLOTS)


def _build(b3val):
    nc = bacc.Bacc("TRN2", target_bir_lowering=False, debug=False)

    z1Tq_d = [
        nc.dram_tensor(f"z1Tq{q}", (EMB, 256), FP16, kind="ExternalInput")
        for q in range(4)
    ]
    z2T_d = nc.dram_tensor("z2T", (EMB, SH), F32, kind="ExternalInput")
    w1aT_d = nc.dram_tensor("w1aT", (EMB, HID), FP16, kind="ExternalInput")
    w1bT_d = nc.dram_tensor("w1bT", (EMB, HID), F32, kind="ExternalInput")
    w2T_d = nc.dram_tensor("w2T", (HID, HID), F32, kind="ExternalInput")
    w3v_d = nc.dram_tensor("w3v", (HID, 1024), F32, kind="ExternalInput")
    b1_d = nc.dram_tensor("b1", (HID,), F32, kind="ExternalInput")
    b2_d = nc.dram_tensor("b2", (HID,), F32, kind="ExternalInput")
    out_d = nc.dram_tensor("out", (SH, N), F32, kind="ExternalOutput")

    with tile.TileContext(nc) as tc:
        _body(tc, out_d, z1Tq_d, z2T_d, w1aT_d, w1bT_d, w2T_d, w3v_d,
              b1_d, b2_d, b3val)

    nc.compile()
    return nc


def _body(tc, out_d, z1Tq_d, z2T_d, w1aT_d, w1bT_d, w2T_d, w3v_d,
          b1_d, b2_d, b3val):
    nc = tc.nc
    with (
        tc.tile_pool(name="const", bufs=1) as const,
        tc.tile_pool(name="h1p", bufs=4) as h1p,
        tc.tile_pool(name="h2p", bufs=3) as h2p,
        tc.tile_pool(name="srows", bufs=1) as srows,
        tc.tile_pool(name="ringp", bufs=1, space="PSUM") as ringp,
    ):
        # ACT warms the gelu table as its very first instruction (no DMAs
        # ride the scalar queue at startup).
        tiny = const.tile([1, 1], F32)
        nc.vector.memset(tiny, 0.0)
        warm = const.tile([1, 1], F32)
        nc.scalar.activation(warm, tiny, GELU)

        # ---- input DMAs spread over sync / scalar / gpsimd queues ----
        w1aT_sb = const.tile([128, HID], FP16)
        nc.sync.dma_start(out=w1aT_sb, in_=w1aT_d.ap())
        w1bT_sb = const.tile([128, HID], F32)
        nc.scalar.dma_start(out=w1bT_sb, in_=w1bT_d.ap())
        z2T_sb = const.tile([128, SH], F32)
        nc.sync.dma_start(out=z2T_sb, in_=z2T_d.ap())
        z1T_sb = const.tile([128, N], FP16)
        for q, eng in enumerate((nc.sync, nc.scalar, nc.gpsimd, nc.scalar)):
            eng.dma_start(out=z1T_sb[:, q * 256:(q + 1) * 256], in_=z1Tq_d[q].ap())
        w2T_f = const.tile([128, HID], F32)
        nc.sync.dma_start(out=w2T_f, in_=w2T_d.ap())
        w3v_f = const.tile([128, 1024], F32)
        nc.gpsimd.dma_start(out=w3v_f, in_=w3v_d.ap())
        b1_sb = const.tile([128, 1], F32)
        nc.gpsimd.dma_start(out=b1_sb, in_=b1_d.ap().rearrange("(p o) -> p o", o=1))
        b2_sb = const.tile([128, 1], F32)
        nc.gpsimd.dma_start(out=b2_sb, in_=b2_d.ap().rearrange("(p o) -> p o", o=1))

        c3p_sb = const.tile([128, 1], F32)
        nc.gpsimd.memset(c3p_sb, CT3)

        # fp16 stationaries (w2T on the startup-idle ACT, w3v on Pool)
        w2T_full = const.tile([128, HID], FP16)
        nc.scalar.activation(w2T_full, w2T_f, COPY, bias=0.0)
        w2T_half = const.tile([128, HID], FP16)  # x(0.5/SQB): h1' = SQB*2*gelu
        nc.scalar.activation(w2T_half, w2T_f, COPY, bias=0.0, scale=0.5 / SQB)
        w3v_h = const.tile([128, 1024], FP16)
        nc.gpsimd.tensor_copy(w3v_h, w3v_f)

        # ---- PSUM: 3 v-slots + [128,1024] W3 accumulation block ----
        ring = ringp.tile([128, 4096], F32)
        VS = [ring[:, 0:1024], ring[:, 1024:2048], ring[:, 2048:3072]]
        w3blk = ring[:, 3072:4096]

        # ---- prologue: b_pp tiles, scaled duplicated a ----
        tpb = ring[:, 2048:2048 + SH]   # v-slot 2 region, freed before use
        nc.tensor.matmul(tpb, w1bT_sb, z2T_sb)
        b_pp_sc = const.tile([128, SH], F32)       # SQB*(b + b1)
        nc.vector.tensor_scalar(out=b_pp_sc, in0=tpb, scalar1=b1_sb[:, 0:1],
                                scalar2=SQB, op0=mybir.AluOpType.add,
                                op1=mybir.AluOpType.mult)
        b_pp = const.tile([128, SH], F32)          # b + b1 (ACT slots)
        nc.vector.tensor_scalar(out=b_pp, in0=tpb, scalar1=b1_sb[:, 0:1],
                                scalar2=None, op0=mybir.AluOpType.add)
        d_sc = const.tile([128, SH], F32)          # pair bias deltas
        nc.vector.tensor_tensor(out=d_sc[:, 0:SH - 1], in0=b_pp_sc[:, 1:SH],
                                in1=b_pp_sc[:, 0:SH - 1],
                                op=mybir.AluOpType.subtract)

        tpa = ring[:, 0:1024]
        for q in range(4):
            nc.tensor.matmul(tpa[:, q * 256:(q + 1) * 256], w1aT_sb,
                             z1T_sb[:, q * 256:(q + 1) * 256])
        a_dbl = const.tile([128, 2048], F32)       # SQB*a, twice
        nc.vector.tensor_scalar(out=a_dbl[:, 0:1024], in0=tpa, scalar1=SQB,
                                scalar2=None, op0=mybir.AluOpType.mult)
        nc.scalar.activation(a_dbl[:, 1024:2048], tpa, COPY, bias=0.0,
                             scale=SQB)

        # ---- steady state ----
        srow = srows.tile([128, N], F32)
        h1map = {}

        def pump_g1(upto):
            s = pump_g1.next
            while s < min(upto, SH):
                if s in ACT_SLOTS:
                    h1 = h1p.tile([128, N], FP16, tag="h1s", name="h1s", bufs=3)
                    nc.scalar.activation(h1, a_dbl[:, 0:1024], GELU,
                                         bias=b_pp[:, s:s + 1], scale=1.0 / SQB)
                    h1map[s] = (h1, 0)
                    s += 1
                elif s + 1 < SH and (s + 1) not in ACT_SLOTS:
                    h1 = h1p.tile([128, 2048], FP16, tag="h1d", name="h1d",
                                  bufs=6)
                    nc.vector._custom_dve(
                        GELU1P_OP,
                        out=h1[:, :].rearrange("p (s n) -> p s n", n=N),
                        in0=a_dbl[:, :].rearrange("p (s n) -> p s n", n=N),
                        in1=c3p_sb[:, 0:1],
                        s0=b_pp_sc[:, s:s + 1], s1=d_sc[:, s:s + 1], imm2=CT2)
                    h1map[s] = (h1, 0)
                    h1map[s + 1] = (h1, 1024)
                    s += 2
                else:
                    h1 = h1p.tile([128, N], FP16, tag="h1s", name="h1s", bufs=3)
                    nc.vector._custom_dve(
                        GELU1_OP, out=h1, in0=a_dbl[:, 0:1024],
                        in1=c3p_sb[:, 0:1],
                        s0=b_pp_sc[:, s:s + 1], s1=1.0, imm2=CT2)
                    h1map[s] = (h1, 0)
                    s += 1
            pump_g1.next = s

        pump_g1.next = 0

        def emit_w2(s):
            h1, off = h1map.pop(s)
            w2 = w2T_full if s in ACT_SLOTS else w2T_half
            vs = VS[s % 3]
            for h in range(2):
                nc.tensor.matmul(vs[:, h * 512:(h + 1) * 512], w2,
                                 h1[:, off + h * 512:off + (h + 1) * 512])

        def emit_w3(s, h2, off):
            g, k = divmod(s, 32)
            w3k = w3v_h[:, 32 * k:32 * k + 32]
            for h in range(2):
                nc.tensor.matmul(
                    w3blk[32 * g:32 * g + 32, h * 512:(h + 1) * 512],
                    w3k, h2[:, off + h * 512:off + (h + 1) * 512],
                    tile_position=(0, 32 * g),
                    start=(k == 0), stop=(k == 31), skip_group_check=True)

        h2q = []

        def emit_g2_pair(s0, s1):
            h2 = h2p.tile([128, 2048], FP16, tag="h2", name="h2", bufs=5)
            nc.scalar.activation(h2, ring[:, (s0 % 3) * 1024:(s0 % 3) * 1024 + 2048],
                                 GELU, bias=b2_sb[:, 0:1])
            h2q.append((s0, h2, 0))
            h2q.append((s1, h2, 1024))

        def emit_g2_single(s):
            h2 = h2p.tile([128, 1024], FP16, tag="h2s", name="h2s", bufs=4)
            nc.scalar.activation(h2, VS[s % 3], GELU, bias=b2_sb[:, 0:1])
            h2q.append((s, h2, 0))

        def evac(g):
            # g==2: groups 0-2 in one pass (partition count is free);
            # g==3: final group, lands in the drain tail.
            if g < 2:
                return
            lo, hi = (0, 96) if g == 2 else (96, 128)
            nc.vector.tensor_scalar(
                out=srow[lo:hi, :], in0=w3blk[lo:hi, :],
                scalar1=b3val, scalar2=None, op0=mybir.AluOpType.add)
            nc.sync.dma_start(out=out_d.ap()[lo:hi, :], in_=srow[lo:hi, :])

        next_evac = 0
        w3_done = -1
        for s in range(SH):
            pump_g1(s + 6)
            emit_w2(s)
            r = s % 3
            if r == 1:
                emit_g2_pair(s - 1, s)
            elif r == 2:
                emit_g2_single(s)
            # W3 batched after the W2 legs of the next pair (see docstring)
            if r == 1:
                while h2q and h2q[0][0] <= s - 3:
                    sl, h2, off = h2q.pop(0)
                    emit_w3(sl, h2, off)
                    w3_done = sl
            while w3_done >= 32 * next_evac + 31:
                evac(next_evac)
                next_evac += 1
        while h2q:
            sl, h2, off = h2q.pop(0)
            emit_w3(sl, h2, off)
            w3_done = sl
            while w3_done >= 32 * next_evac + 31:
                evac(next_evac)
                next_evac += 1


_NC_CACHE = {}


def make_in_maps(z1, z2, W1, b1, W2, b2, W3, b3):
    f = np.float32
    z1 = np.asarray(z1, dtype=f)
    z2 = np.asarray(z2, dtype=f)
    W1 = np.asarray(W1, dtype=f)
    b1 = np.ascontiguousarray(np.asarray(b1, dtype=f))
    W2 = np.asarray(W2, dtype=f)
    b2 = np.ascontiguousarray(np.asarray(b2, dtype=f))
    W3 = np.asarray(W3, dtype=f)
    b3 = np.ascontiguousarray(np.asarray(b3, dtype=f))

    z1T = np.ascontiguousarray(z1.T.astype(np.float16))
    z1Tq = {
        f"z1Tq{q}": np.ascontiguousarray(z1T[:, q * 256:(q + 1) * 256])
        for q in range(4)
    }
    w1aT = np.ascontiguousarray(W1[:, :EMB].T.astype(np.float16))
    w1bT = np.ascontiguousarray(W1[:, EMB:].T)
    w2T = np.ascontiguousarray(W2.T)
    w3v = np.zeros((HID, 1024), dtype=f)
    for k in range(32):
        w3v[:, 32 * k + k] = W3[0]

    # slot permutation: widest-range i's (per core) -> ACT_SLOTS
    a = z1 @ W1[:, :EMB].T            # host copy, scheduling only
    bb = z2 @ W1[:, EMB:].T + b1
    amin, amax = a.min(0), a.max(0)
    ximax = np.maximum(np.abs(amin[None, :] + bb), np.abs(amax[None, :] + bb)).max(1)

    maps, perms = [], []
    act_slots = list(ACT_SLOTS)
    dve_slots = list(DVE_SLOTS)
    for c in range(NCORES):
        loc = ximax[c * SH:(c + 1) * SH]
        order = np.argsort(-loc)
        perm = np.empty(SH, dtype=np.int64)
        perm[act_slots] = order[:len(act_slots)]
        perm[dve_slots] = order[len(act_slots):]
        perms.append(perm)
        z2c = z2[c * SH:(c + 1) * SH][perm]
        maps.append({
            **z1Tq,
            "z2T": np.ascontiguousarray(z2c.T),
            "w1aT": w1aT, "w1bT": w1bT, "w2T": w2T, "w3v": w3v,
            "b1": b1, "b2": b2,
        })
    return maps, perms


def gather_out(results, perms):
    blocks = []
    for c in range(NCORES):
        o = results[c]["out"]
        inv = np.empty(SH, dtype=np.int64)
        inv[perms[c]] = np.arange(SH)
        blocks.append(o[inv])
    return np.concatenate(blocks, axis=0)


def kernel(z1, z2, W1, b1, W2, b2, W3, b3):
    b3v = float(np.asarray(b3).reshape(-1)[0])
    key = round(b3v, 9)
    if key not in _NC_CACHE:
        _NC_CACHE[key] = _build(b3v)
    nc = _NC_CACHE[key]

    in_maps, perms = make_in_maps(z1, z2, W1, b1, W2, b2, W3, b3)
    res = bass_utils.run_bass_kernel_spmd(nc, in_maps, core_ids=list(range(NCORES)))
    return gather_out(res.results, perms)


if __name__ == "__main__":
    rng = np.random.default_rng(0)
    s1 = 1.0 / np.sqrt(2 * EMB)
    s2 = 1.0 / np.sqrt(HID)
    ins = dict(
        z1=rng.standard_normal((N, EMB), dtype=np.float32),
        z2=rng.standard_normal((N, EMB), dtype=np.float32),
        W1=rng.uniform(-s1, s1, (HID, 2 * EMB)).astype(np.float32),
        b1=rng.uniform(-s1, s1, (HID,)).astype(np.float32),
        W2=rng.uniform(-s2, s2, (HID, HID)).astype(np.float32),
        b2=rng.uniform(-s2, s2, (HID,)).astype(np.float32),
        W3=rng.uniform(-s2, s2, (1, HID)).astype(np.float32),
        b3=rng.uniform(-s2, s2, (1,)).astype(np.float32),
    )
    out = kernel(**ins)
    print("out", out.shape, out.dtype, out[:2, :4])


# revision 15
# speedup vs baseline: 1.0403x; 1.0019x over previous
"""Fused pairwise-MLP kernel for Trainium2 (8 NeuronCores, SPMD data-parallel).

Computes log_q[i, j] = W3 @ gelu(W2 @ gelu(a[j] + b[i] + b1) + b2) + b3
with a = z1 @ W1a.T, b = z2 @ W1b.T  (W1 = [W1a | W1b]), N=1024, H=EMB=128.

Sharding: rows of i (z2) split across 8 cores, z1 + weights replicated
(host-side sharding; no collectives).

The two gelu passes (2 x 131072 128-partition columns per core) are the
arithmetic bottleneck: the ACT engine runs them at 1 elem/lane/cycle and
nothing else on the chip has a gelu table.  gelu1 therefore runs almost
entirely on the Vector engine as a runtime-registered custom DVE op that
evaluates, in one 8-ALU-stage pass over two slots (2048 cols, subdim
form, per-slot bias via PageIdx),
    y = x' + ((x'^2 + ct2)*x'^2 + ct3)*x'^2,   x' = sqrt(beta)*(a + bias)
which equals sqrt(beta)*2*gelu(x) for a beta-normalized deg-6 even fit
(the leading Horner coefficient is normalized to 1 so the three scalar
ports cover bias-step + two coefficients).  The W2 stationary for these
slots is pre-scaled by 0.5/sqrt(beta).  The NA widest-range slots per
core run on ACT's exact gelu (host permutes i-rows so they land on the
fixed ACT slot positions; permutation undone on output gather).

gelu2 runs entirely on ACT (PSUM input, b2 via the bias port) as
2048-wide pair instructions + a 1024 single per 3-slot PSUM v-ring
period.  W3 matmuls are emitted 3+ slots late and batched after the
next period's W2 legs so the in-order PE stream never head-of-line
blocks the v-ring round-trip (pair-gelu2 -> 2x W2 -> next pair).

The W3 dot uses 32 zero-padded stationary variants (w3 at column k) so
slot s = 32g+k lands at PSUM partition s of a single [128,1024]
accumulation block (tile_position group g, accumulating matmuls).  All
128 output rows are evacuated with 4 instructions (+b3) and 5 DMAs.
"""

import numpy as np

import concourse.bacc as bacc
import concourse.bass as bass
import concourse.tile as tile
import concourse.mybir as mybir
from concourse import bass_utils

import concourse.dve_ops as dve_ops
from concourse.dve_ops import DveOp, OPS
from concourse.dve_spec import (
    Spec, Src0, C0, C1, C2, C3, PageIdx, lower, _spill_c3_to_src1,
)
from concourse.dve_uop import DveOpSpec


def _register(name, spec, subdim):
    if name in dve_ops._SUB_OPCODE_FOR_NAME:
        return next(o for o in OPS if o.name == name)
    row = dve_ops._CUSTOM_DVE_ROW_BASE + len(OPS)
    dve_ops._SUB_OPCODE_FOR_NAME[name] = row
    shas = {}
    for ver in ("v3", "v4"):
        try:
            s = DveOpSpec(name=name, opcode=row, uops=lower(spec, ver=ver),
                          rd1_en=True)
            shas[ver] = s.sha(ver)
        except Exception:
            pass
    op = DveOp(name, spec, subdim=subdim, uops_sha=shas)
    OPS.append(op)
    dve_ops.CUSTOM_DVE_SPECS[name] = spec
    return op


def _gelu1_single_spec():
    # x = in0 + s0; u = x*x; out = ((s1*u + imm2)*u + c3)*u + x
    x = Src0 + C0
    u = x * x
    t = ((C1 * u + C2) * u + C3) * u
    body = _spill_c3_to_src1(t + x)

    def ref(in0, in1, s0, s1, imm2):
        xx = in0.astype(np.float32) + s0
        uu = xx * xx
        return ((s1 * uu + imm2) * uu + in1) * uu + xx

    return Spec(body=body, reference=ref)


def _gelu1_pair_spec():
    # in0 = [P, 2, N]; bias steps via PageIdx(C0, C1); leading coef = 1
    pg = PageIdx(C0, C1)
    x = Src0 + pg
    u = x * x
    t = ((u + C2) * u + C3) * u
    body = _spill_c3_to_src1(t + x)

    def ref(in0, in1, s0, s1, imm2):
        x = in0.astype(np.float32)
        S = int(np.prod(x.shape[1:-1]))
        x3 = x.reshape((x.shape[0], S, x.shape[-1]))
        idx = s0[:, None] if isinstance(s0, np.ndarray) else s0
        s1v = s1[:, None] if isinstance(s1, np.ndarray) else s1
        bias = idx + np.arange(S, dtype=np.float32)[None, :, None] * s1v
        xx = x3 + bias
        uu = xx * xx
        c3v = in1 if not isinstance(in1, np.ndarray) else in1.reshape(-1, 1, 1)
        y = ((uu + imm2) * uu + c3v) * uu + xx
        return y.reshape(in0.shape)

    return Spec(body=body, reference=ref)


GELU1_OP = _register("GELU1_EVEN6_ANT", _gelu1_single_spec(), subdim=False)
GELU1P_OP = _register("GELU1_PAIR_ANT", _gelu1_pair_spec(), subdim=True)

# deg-6 even fit of x*erf(x/sqrt(2)) on |x| <= 3.67 (density-weighted,
# x ~ N(0, 0.586)): coefs of u, u^2, u^3
CC = (0.7720335236204651, -0.09365603610221726, 0.00457457167839083)
BETA = CC[2] ** 0.4               # leading-coef normalization
SQB = float(np.sqrt(BETA))
CT2 = float(CC[1] / BETA ** 1.5)  # u'^2 coef after normalization
CT3 = float(CC[0] / SQB)          # u'^1 coef

N = 1024
EMB = 128
HID = 128
NCORES = 8
SH = N // NCORES  # i-slots per core
F32 = mybir.dt.float32
FP16 = mybir.dt.float16
GELU = mybir.ActivationFunctionType.Gelu
COPY = mybir.ActivationFunctionType.Copy

# slots whose gelu1 runs on ACT (exact); host routes widest-range i's
# here.  Gaps of 15 keep every DVE run even-length (pairable).
ACT_SLOTS = tuple(range(6, 112, 15))  # 8 slots: 6,21,...,111
DVE_SLOTS = tuple(s for s in range(SH) if s not in ACT_SLOTS)


def _build(b3val):
    nc = bacc.Bacc("TRN2", target_bir_lowering=False, debug=False)

    z1Tq_d = [
        nc.dram_tensor(f"z1Tq{q}", (EMB, 256), FP16, kind="ExternalInput")
        for q in range(4)
    ]
    z2T_d = nc.dram_tensor("z2T", (EMB, SH), F32, kind="ExternalInput")
    w1aT_d = nc.dram_tensor("w1aT", (EMB, HID), FP16, kind="ExternalInput")
    w1bT_d = nc.dram_tensor("w1bT", (EMB, HID), F32, kind="ExternalInput")
    w2T_d = nc.dram_tensor("w2T", (HID, HID), F32, kind="ExternalInput")
    w3v_d = nc.dram_tensor("w3v", (HID, 1024), F32, kind="ExternalInput")
    b1_d = nc.dram_tensor("b1", (HID,), F32, kind="ExternalInput")
    b2_d = nc.dram_tensor("b2", (HID,), F32, kind="ExternalInput")
    out_d = nc.dram_tensor("out", (SH, N), F32, kind="ExternalOutput")

    with tile.TileContext(nc) as tc:
        _body(tc, out_d, z1Tq_d, z2T_d, w1aT_d, w1bT_d, w2T_d, w3v_d,
              b1_d, b2_d, b3val)

    nc.compile()
    return nc


def _body(tc, out_d, z1Tq_d, z2T_d, w1aT_d, w1bT_d, w2T_d, w3v_d,
          b1_d, b2_d, b3val):
    nc = tc.nc
    with (
        tc.tile_pool(name="const", bufs=1) as const,
        tc.tile_pool(name="h1p", bufs=4) as h1p,
        tc.tile_pool(name="h2p", bufs=3) as h2p,
        tc.tile_pool(name="srows", bufs=1) as srows,
        tc.tile_pool(name="ringp", bufs=1, space="PSUM") as ringp,
    ):
        # ACT warms the gelu table as its very first instruction (no DMAs
        # ride the scalar queue at startup).
        tiny = const.tile([1, 1], F32)
        nc.vector.memset(tiny, 0.0)
        warm = const.tile([1, 1], F32)
        nc.scalar.activation(warm, tiny, GELU)

        # ---- input DMAs: z1T quarters lead the HWDGE queues ----
        z1T_sb = const.tile([128, N], FP16)
        for q, eng in enumerate((nc.sync, nc.scalar, nc.sync, nc.scalar)):
            eng.dma_start(out=z1T_sb[:, q * 256:(q + 1) * 256], in_=z1Tq_d[q].ap())
        w1aT_sb = const.tile([128, HID], FP16)
        nc.gpsimd.dma_start(out=w1aT_sb, in_=w1aT_d.ap())
        w1bT_sb = const.tile([128, HID], F32)
        nc.gpsimd.dma_start(out=w1bT_sb, in_=w1bT_d.ap())
        z2T_sb = const.tile([128, SH], F32)
        nc.sync.dma_start(out=z2T_sb, in_=z2T_d.ap())
        w2T_f = const.tile([128, HID], F32)
        nc.scalar.dma_start(out=w2T_f, in_=w2T_d.ap())
        w3v_f = const.tile([128, 1024], F32)
        nc.gpsimd.dma_start(out=w3v_f, in_=w3v_d.ap())
        b1_sb = const.tile([128, 1], F32)
        nc.gpsimd.dma_start(out=b1_sb, in_=b1_d.ap().rearrange("(p o) -> p o", o=1))
        b2_sb = const.tile([128, 1], F32)
        nc.gpsimd.dma_start(out=b2_sb, in_=b2_d.ap().rearrange("(p o) -> p o", o=1))

        c3p_sb = const.tile([128, 1], F32)
        nc.gpsimd.memset(c3p_sb, CT3)

        # fp16 stationaries (w2T on the startup-idle ACT, w3v on Pool)
        w2T_full = const.tile([128, HID], FP16)
        nc.scalar.activation(w2T_full, w2T_f, COPY, bias=0.0)
        w2T_half = const.tile([128, HID], FP16)  # x(0.5/SQB): h1' = SQB*2*gelu
        nc.scalar.activation(w2T_half, w2T_f, COPY, bias=0.0, scale=0.5 / SQB)
        w3v_h = const.tile([128, 1024], FP16)
        nc.gpsimd.tensor_copy(w3v_h, w3v_f)

        # ---- PSUM: 3 v-slots + [128,1024] W3 accumulation block ----
        ring = ringp.tile([128, 4096], F32)
        VS = [ring[:, 0:1024], ring[:, 1024:2048], ring[:, 2048:3072]]
        w3blk = ring[:, 3072:4096]

        # ---- prologue: b_pp tiles, scaled duplicated a ----
        tpb = ring[:, 2048:2048 + SH]   # v-slot 2 region, freed before use
        nc.tensor.matmul(tpb, w1bT_sb, z2T_sb)
        b_pp_sc = const.tile([128, SH], F32)       # SQB*(b + b1)
        nc.vector.tensor_scalar(out=b_pp_sc, in0=tpb, scalar1=b1_sb[:, 0:1],
                                scalar2=SQB, op0=mybir.AluOpType.add,
                                op1=mybir.AluOpType.mult)
        b_pp = const.tile([128, SH], F32)          # b + b1 (ACT slots)
        nc.vector.tensor_scalar(out=b_pp, in0=tpb, scalar1=b1_sb[:, 0:1],
                                scalar2=None, op0=mybir.AluOpType.add)
        d_sc = const.tile([128, SH], F32)          # pair bias deltas
        nc.vector.tensor_tensor(out=d_sc[:, 0:SH - 1], in0=b_pp_sc[:, 1:SH],
                                in1=b_pp_sc[:, 0:SH - 1],
                                op=mybir.AluOpType.subtract)

        tpa = ring[:, 0:1024]
        for q in range(4):
            nc.tensor.matmul(tpa[:, q * 256:(q + 1) * 256], w1aT_sb,
                             z1T_sb[:, q * 256:(q + 1) * 256])
        a_dbl = const.tile([128, 2048], F32)       # SQB*a, twice
        nc.vector.tensor_scalar(out=a_dbl[:, 0:1024], in0=tpa, scalar1=SQB,
                                scalar2=None, op0=mybir.AluOpType.mult)
        nc.scalar.activation(a_dbl[:, 1024:2048], tpa, COPY, bias=0.0,
                             scale=SQB)

        # ---- steady state ----
        srow = srows.tile([128, N], F32)
        h1map = {}

        def pump_g1(upto):
            s = pump_g1.next
            while s < min(upto, SH):
                if s in ACT_SLOTS:
                    h1 = h1p.tile([128, N], FP16, tag="h1s", name="h1s", bufs=3)
                    nc.scalar.activation(h1, a_dbl[:, 0:1024], GELU,
                                         bias=b_pp[:, s:s + 1], scale=1.0 / SQB)
                    h1map[s] = (h1, 0)
                    s += 1
                elif s + 1 < SH and (s + 1) not in ACT_SLOTS:
                    h1 = h1p.tile([128, 2048], FP16, tag="h1d", name="h1d",
                                  bufs=6)
                    nc.vector._custom_dve(
                        GELU1P_OP,
                        out=h1[:, :].rearrange("p (s n) -> p s n", n=N),
                        in0=a_dbl[:, :].rearrange("p (s n) -> p s n", n=N),
                        in1=c3p_sb[:, 0:1],
                        s0=b_pp_sc[:, s:s + 1], s1=d_sc[:, s:s + 1], imm2=CT2)
                    h1map[s] = (h1, 0)
                    h1map[s + 1] = (h1, 1024)
                    s += 2
                else:
                    h1 = h1p.tile([128, N], FP16, tag="h1s", name="h1s", bufs=3)
                    nc.vector._custom_dve(
                        GELU1_OP, out=h1, in0=a_dbl[:, 0:1024],
                        in1=c3p_sb[:, 0:1],
                        s0=b_pp_sc[:, s:s + 1], s1=1.0, imm2=CT2)
                    h1map[s] = (h1, 0)
                    s += 1
            pump_g1.next = s

        pump_g1.next = 0

        def emit_w2(s):
            h1, off = h1map.pop(s)
            w2 = w2T_full if s in ACT_SLOTS else w2T_half
            vs = VS[s % 3]
            for h in range(2):
                nc.tensor.matmul(vs[:, h * 512:(h + 1) * 512], w2,
                                 h1[:, off + h * 512:off + (h + 1) * 512])

        def emit_w3(s, h2, off):
            g, k = divmod(s, 32)
            w3k = w3v_h[:, 32 * k:32 * k + 32]
            for h in range(2):
                nc.tensor.matmul(
                    w3blk[32 * g:32 * g + 32, h * 512:(h + 1) * 512],
                    w3k, h2[:, off + h * 512:off + (h + 1) * 512],
                    tile_position=(0, 32 * g),
                    start=(k == 0), stop=(k == 31), skip_group_check=True)

        h2q = []

        def emit_g2_pair(s0, s1):
            h2 = h2p.tile([128, 2048], FP16, tag="h2", name="h2", bufs=5)
            nc.scalar.activation(h2, ring[:, (s0 % 3) * 1024:(s0 % 3) * 1024 + 2048],
                                 GELU, bias=b2_sb[:, 0:1])
            h2q.append((s0, h2, 0))
            h2q.append((s1, h2, 1024))

        def emit_g2_single(s):
            h2 = h2p.tile([128, 1024], FP16, tag="h2s", name="h2s", bufs=4)
            nc.scalar.activation(h2, VS[s % 3], GELU, bias=b2_sb[:, 0:1])
            h2q.append((s, h2, 0))

        def evac(g):
            # g==2: groups 0-2 in one pass (partition count is free);
            # g==3: final group, lands in the drain tail.
            if g < 2:
                return
            lo, hi = (0, 96) if g == 2 else (96, 128)
            nc.vector.tensor_scalar(
                out=srow[lo:hi, :], in0=w3blk[lo:hi, :],
                scalar1=b3val, scalar2=None, op0=mybir.AluOpType.add)
            nc.sync.dma_start(out=out_d.ap()[lo:hi, :], in_=srow[lo:hi, :])

        next_evac = 0
        w3_done = -1
        for s in range(SH):
            pump_g1(s + 6)
            emit_w2(s)
            r = s % 3
            if r == 1:
                emit_g2_pair(s - 1, s)
            elif r == 2:
                emit_g2_single(s)
            # W3 batched after the W2 legs of the next pair (see docstring)
            if r == 1:
                while h2q and h2q[0][0] <= s - 3:
                    sl, h2, off = h2q.pop(0)
                    emit_w3(sl, h2, off)
                    w3_done = sl
            while w3_done >= 32 * next_evac + 31:
                evac(next_evac)
                next_evac += 1
        while h2q:
            sl, h2, off = h2q.pop(0)
            emit_w3(sl, h2, off)
            w3_done = sl
            while w3_done >= 32 * next_evac + 31:
                evac(next_evac)
                next_evac += 1


_NC_CACHE = {}


def make_in_maps(z1, z2, W1, b1, W2, b2, W3, b3):
    f = np.float32
    z1 = np.asarray(z1, dtype=f)
    z2 = np.asarray(z2, dtype=f)
    W1 = np.asarray(W1, dtype=f)
    b1 = np.ascontiguousarray(np.asarray(b1, dtype=f))
    W2 = np.asarray(W2, dtype=f)
    b2 = np.ascontiguousarray(np.asarray(b2, dtype=f))
    W3 = np.asarray(W3, dtype=f)
    b3 = np.ascontiguousarray(np.asarray(b3, dtype=f))

    z1T = np.ascontiguousarray(z1.T.astype(np.float16))
    z1Tq = {
        f"z1Tq{q}": np.ascontiguousarray(z1T[:, q * 256:(q + 1) * 256])
        for q in range(4)
    }
    w1aT = np.ascontiguousarray(W1[:, :EMB].T.astype(np.float16))
    w1bT = np.ascontiguousarray(W1[:, EMB:].T)
    w2T = np.ascontiguousarray(W2.T)
    w3v = np.zeros((HID, 1024), dtype=f)
    for k in range(32):
        w3v[:, 32 * k + k] = W3[0]

    # slot permutation: widest-range i's (per core) -> ACT_SLOTS
    a = z1 @ W1[:, :EMB].T            # host copy, scheduling only
    bb = z2 @ W1[:, EMB:].T + b1
    amin, amax = a.min(0), a.max(0)
    ximax = np.maximum(np.abs(amin[None, :] + bb), np.abs(amax[None, :] + bb)).max(1)

    maps, perms = [], []
    act_slots = list(ACT_SLOTS)
    dve_slots = list(DVE_SLOTS)
    for c in range(NCORES):
        loc = ximax[c * SH:(c + 1) * SH]
        order = np.argsort(-loc)
        perm = np.empty(SH, dtype=np.int64)
        perm[act_slots] = order[:len(act_slots)]
        perm[dve_slots] = order[len(act_slots):]
        perms.append(perm)
        z2c = z2[c * SH:(c + 1) * SH][perm]
        maps.append({
            **z1Tq,
            "z2T": np.ascontiguousarray(z2c.T),
            "w1aT": w1aT, "w1bT": w1bT, "w2T": w2T, "w3v": w3v,
            "b1": b1, "b2": b2,
        })
    return maps, perms


def gather_out(results, perms):
    blocks = []
    for c in range(NCORES):
        o = results[c]["out"]
        inv = np.empty(SH, dtype=np.int64)
        inv[perms[c]] = np.arange(SH)
        blocks.append(o[inv])
    return np.concatenate(blocks, axis=0)


def kernel(z1, z2, W1, b1, W2, b2, W3, b3):
    b3v = float(np.asarray(b3).reshape(-1)[0])
    key = round(b3v, 9)
    if key not in _NC_CACHE:
        _NC_CACHE[key] = _build(b3v)
    nc = _NC_CACHE[key]

    in_maps, perms = make_in_maps(z1, z2, W1, b1, W2, b2, W3, b3)
    res = bass_utils.run_bass_kernel_spmd(nc, in_maps, core_ids=list(range(NCORES)))
    return gather_out(res.results, perms)


if __name__ == "__main__":
    rng = np.random.default_rng(0)
    s1 = 1.0 / np.sqrt(2 * EMB)
    s2 = 1.0 / np.sqrt(HID)
    ins = dict(
        z1=rng.standard_normal((N, EMB), dtype=np.float32),
        z2=rng.standard_normal((N, EMB), dtype=np.float32),
        W1=rng.uniform(-s1, s1, (HID, 2 * EMB)).astype(np.float32),
        b1=rng.uniform(-s1, s1, (HID,)).astype(np.float32),
        W2=rng.uniform(-s2, s2, (HID, HID)).astype(np.float32),
        b2=rng.uniform(-s2, s2, (HID,)).astype(np.float32),
        W3=rng.uniform(-s2, s2, (1, HID)).astype(np.float32),
        b3=rng.uniform(-s2, s2, (1,)).astype(np.float32),
    )
    out = kernel(**ins)
    print("out", out.shape, out.dtype, out[:2, :4])
